# revision 29
# baseline (speedup 1.0000x reference)
"""AdaLN-modulated multi-head attention block on 8 TRN2 NeuronCores.

Shapes (hardcoded): B=8, T=1024, D=1024, H=16 heads, e=64 head dim.
Sharding: pure data-parallel - one batch element per core, weights
replicated, no collectives.

v3 design (vs v2 baseline, 590us):
  The v2 trace showed PE busy 75% but with throttle_active=344us: the
  strict phase segregation (V-blocks with idle PE) kept the HAM clock
  gate at K=4/8 for most of the kernel, so matmuls ran at 1.2GHz.
  v3 software-pipelines across head groups so the PE always has
  independent matmul work queued:
    - v blocks + g0 stats/rope interleaved (C phase)
    - g0 attention interleaved with g1 projections/stats/ropes (P1)
    - g1 attention interleaved with g0 normalize + g0 y-partials (P2)
    - tail: g1 normalize + g1 y-partials + gate + out
  All ACT transcendentals use one table set (natural_log_exp):
    rsqrt(v) = exp(-0.5*ln(v+eps)); 1/x = exp(-ln(x));
    silu(x) = x * 1/(1+exp(-x)) (tiny DVE recip)
  This kills ~12 ACT table switches and ~35us of slow DVE reciprocals.
"""

import sys

try:
    import concourse  # noqa: F401  (provided by the environment, e.g. axon_site)
except ImportError:
    sys.path.append("/opt/trn_rl_repo")

import contextlib

import numpy as np

import concourse.bass as bass
import concourse.mybir as mybir
import concourse.tile as tile
from concourse import bacc
from concourse.bass_utils import run_bass_kernel_spmd

F32 = mybir.dt.float32
BF16 = mybir.dt.bfloat16
AF = mybir.ActivationFunctionType
OP = mybir.AluOpType

B, T, D, TD = 8, 1024, 1024, 1024
H, E = 16, 64
P = 128
NT = T // P          # 8 token tiles
ND = D // P          # 8 feature tiles
EPS = 1e-6
N3 = 3 * D
SWAP_MASK = [i ^ 1 for i in range(32)]
PATCH_ACT_TABLES = False


def _patch_act_tables():
    """Make Exp/Ln resolve to the combined natural_log_exp table set.

    The table-load pass maps each activation function to a covering
    set; by default Exp lands in exp_and_others and Ln in natural_log,
    so a kernel mixing them thrashes ACT_TABLE_LOADs (1.3us each).
    Filtering Exp/Ln out of every other set (set ids/order preserved)
    forces both onto natural_log_exp_and_others -> one load total.
    """
    if getattr(bacc, "_act_tables_patched", False):
        return
    orig = bacc.get_activation_tables

    def patched(arch):
        t = orig(arch)
        combo = t.get("natural_log_exp_and_others")
        if combo:
            drop = {f for f in (AF.Exp, AF.Ln) if f in combo}
            for name, fns in t.items():
                if name != "natural_log_exp_and_others":
                    fns.difference_update(drop)
        return t

    bacc.get_activation_tables = patched
    bacc._act_tables_patched = True


def build_nc(apply_qk_weight: bool):
    if PATCH_ACT_TABLES:
        _patch_act_tables()
    nc = bacc.Bacc("TRN2", target_bir_lowering=False, debug=False, num_devices=8)

    aps = {}
    aps["x"] = nc.dram_tensor("x", [T, D], F32, kind="ExternalInput").ap()
    aps["time"] = nc.dram_tensor("time", [TD], F32, kind="ExternalInput").ap()
    # weights arrive host-preconverted to bf16 (halves DMA, no on-chip casts)
    aps["mod_w"] = nc.dram_tensor("mod_w", [TD, N3], BF16, kind="ExternalInput").ap()
    aps["mod_b"] = nc.dram_tensor("mod_b", [N3], F32, kind="ExternalInput").ap()
    aps["w_qkv"] = nc.dram_tensor("w_qkv", [D, N3], BF16, kind="ExternalInput").ap()
    aps["w_out"] = nc.dram_tensor("w_out", [D, D], BF16, kind="ExternalInput").ap()
    # host-precomputed constants
    aps["cs_full"] = nc.dram_tensor("cs_full", [P, T], BF16, kind="ExternalInput").ap()
    aps["sn_full"] = nc.dram_tensor("sn_full", [P, T], BF16, kind="ExternalInput").ap()
    aps["eseg"] = nc.dram_tensor("eseg", [P, ND, 16], BF16, kind="ExternalInput").ap()
    aps["bsegj"] = nc.dram_tensor("bsegj", [8, 4, P], BF16,
                                  kind="ExternalInput").ap()
    aps["ident"] = nc.dram_tensor("ident", [P, P], BF16, kind="ExternalInput").ap()
    aps["ones_row"] = nc.dram_tensor("ones_row", [1, P], BF16, kind="ExternalInput").ap()
    aps["wq_col"] = nc.dram_tensor("wq_col", [P, 1], F32, kind="ExternalInput").ap()
    aps["wk_col"] = nc.dram_tensor("wk_col", [P, 1], F32, kind="ExternalInput").ap()

    aps["out"] = nc.dram_tensor("out", [T, D], F32, kind="ExternalOutput").ap()

    with tile.TileContext(nc) as tc:
        _body(nc, tc, aps, apply_qk_weight)
    nc.finalize()
    return nc


def _body(nc, tc, aps, apply_qk_weight):
    x_e, time_e, modw_e = aps["x"], aps["time"], aps["mod_w"]
    modb_e, wqkv_e, wout_e = aps["mod_b"], aps["w_qkv"], aps["w_out"]
    out_e = aps["out"]

    ctx = contextlib.ExitStack()
    with ctx:
        consts = ctx.enter_context(tc.tile_pool(name="consts", bufs=1))
        big = ctx.enter_context(tc.tile_pool(name="big", bufs=1))
        wstr = ctx.enter_context(tc.tile_pool(name="wstr", bufs=1))
        temps = ctx.enter_context(tc.tile_pool(name="temps", bufs=2))
        small = ctx.enter_context(tc.tile_pool(name="small", bufs=1))
        psum = ctx.enter_context(tc.tile_pool(name="psum", bufs=2, space="PSUM"))

        # PSUM budget (8 banks of [128 x 2KB]):
        #   sc: 2 bufs - score halves
        #   po: 2 bufs - o / v accumulation
        #   pq: 2 bufs - q/k projection chains, transposes, y blocks, bpB
        #   pm: 2 bufs - stats, bpA, mod, gate/norm bcasts
        def ps(tag, shape, name, dtype=F32):
            return psum.tile(shape, dtype, tag=tag, bufs=2, name=name,
                             padded_shape=[P, 512])

        # ---- constants into SBUF -------------------------------------
        def cload(key, shape, dtype, name):
            t = consts.tile(shape, dtype, tag=name, name=name)
            nc.sync.dma_start(t[:], aps[key])
            return t

        ident_sb = cload("ident", [P, P], BF16, "ident_sb")
        eps_sb = consts.tile([P, 1], F32, tag="eps_sb", name="eps_sb")
        nc.gpsimd.memset(eps_sb[:], EPS)

        # ---- big resident tensors ------------------------------------
        hT = big.tile([P, ND, T], BF16, tag="hT", name="hT")       # 16K/part
        qT = big.tile([P, ND, T], BF16, tag="qT", name="qT")       # 16K
        kT = big.tile([P, ND, T], BF16, tag="kT", name="kT")       # 16K
        v_sb = big.tile([P, NT, H, E + 8], BF16, tag="v", name="v_sb")  # 18K
        y0 = big.tile([P, NT, 2, 512], BF16, tag="y0", name="y0")  # 16K
        oTn = qT   # head rows of qT are dead once that head's scores ran

        # ==============================================================
        # DMA prefetch order == PE consumption order:
        # modw(shift+scale) -> x -> wqk g0 -> wv -> wqk g1;
        # gate modw + w_out issued mid-P1.
        # ==============================================================
        def wbig_tile(name):
            return wstr.tile([P, ND, 512], BF16, tag="wbig", bufs=2, name=name)

        # Single sync DMA queue, ordered by need: x first (stage-1
        # streams over it), then mod_w (affine cols), qk weights for B,
        # v weights and rope/stats constants for C, then g1 weights.
        x_tiles = []
        for tt in range(NT):
            xt = temps.tile([P, D], F32, tag="xt", bufs=2, name=f"xt{tt}")
            nc.sync.dma_start(xt[:], x_e[tt * P:(tt + 1) * P, :])
            x_tiles.append(xt)

        modw_sb = {}
        for g, n2 in ((0, 0), (0, 1), (1, 0), (1, 1)):
            mw = wbig_tile(f"modw{g}_{n2}")
            col0 = g * D + n2 * 512
            nc.sync.dma_start(
                mw[:], modw_e[:, col0:col0 + 512].rearrange(
                    "(kc p) j -> p kc j", p=P))
            modw_sb[(g, n2)] = mw

        t8 = small.tile([P, TD // P], F32, tag="t8", name="t8")
        nc.sync.dma_start(t8[:], time_e.rearrange("(o p) -> p o", p=P))
        mrowf_sb = []
        for g in range(2):
            mrowf = temps.tile([1, D], F32, tag="rbc", bufs=2,
                               name=f"mrowf{g}")
            nc.sync.dma_start(mrowf[:], modb_e[None, g * D:(g + 1) * D])
            mrowf_sb.append(mrowf)

        g0_jcs = [jc for jt in range(4) for jc in (jt, ND + jt)]
        g1_jcs = [jc for jt in range(4, 8) for jc in (jt, ND + jt)]
        wqk_sb = {}
        for jc in g0_jcs:
            wt = wstr.tile([P, ND, P], BF16, tag="wqk", bufs=6,
                           name=f"wqk{jc}")
            nc.sync.dma_start(
                wt[:], wqkv_e[:, jc * P:(jc + 1) * P].rearrange(
                    "(kc p) j -> p kc j", p=P))
            wqk_sb[jc] = wt

        wv_sb = []
        for nv in range(2):
            wv = wbig_tile(f"wv{nv}")
            nc.sync.dma_start(
                wv[:], wqkv_e[:, 2048 + nv * 512:2048 + (nv + 1) * 512]
                .rearrange("(kc p) j -> p kc j", p=P))
            wv_sb.append(wv)

        cs_sb = cload("cs_full", [P, T], BF16, "cs_sb")
        sn_sb = cload("sn_full", [P, T], BF16, "sn_sb")
        eseg_sb = cload("eseg", [P, ND, 16], BF16, "eseg_sb")
        bsegj_all = cload("bsegj", [8, 4, P], BF16, "bsegj_sb")
        bsegj_sb = [bsegj_all[:, j, :] for j in range(4)]
        ones_sb = cload("ones_row", [1, P], BF16, "ones_sb")
        wq_sb = cload("wq_col", [P, 1], F32, "wq_sb")
        wk_sb = cload("wk_col", [P, 1], F32, "wk_sb")

        for jc in g1_jcs:
            wt = wstr.tile([P, ND, P], BF16, tag="wqk", bufs=6,
                           name=f"wqk{jc}")
            nc.sync.dma_start(
                wt[:], wqkv_e[:, jc * P:(jc + 1) * P].rearrange(
                    "(kc p) j -> p kc j", p=P))
            wqk_sb[jc] = wt

        # ==============================================================
        # Stage 0: mod = silu(time) @ mod_w + mod_b (shift/scale now).
        # silu = x * 1/(1+exp(-x)) - stays in the exp table set.
        # ==============================================================
        ex8 = small.tile([P, TD // P], F32, tag="ex8", name="ex8")
        nc.scalar.activation(ex8[:], t8[:], AF.Exp, scale=-1.0)
        nc.vector.tensor_scalar_add(ex8[:], ex8[:], 1.0)
        r8 = small.tile([P, TD // P], F32, tag="r8", name="r8")
        nc.vector.reciprocal(r8[:], ex8[:])
        silu8 = small.tile([P, TD // P], BF16, tag="silu8", name="silu8")
        nc.vector.tensor_mul(silu8[:], t8[:], r8[:])

        def mod_group(g, mw_pair):
            """Compute mod group g -> [1, D] bf16 row (bias added)."""
            if g < 2:
                mrowf = mrowf_sb[g]
            else:
                mrowf = temps.tile([1, D], F32, tag="rbc", bufs=2,
                                   name=f"mrowf{g}")
                nc.sync.dma_start(mrowf[:], modb_e[None, g * D:(g + 1) * D])
            mrow = small.tile([1, D], BF16, tag="mrow", bufs=1,
                              name=f"mrow{g}")
            for n2 in range(2):
                mw = mw_pair[n2]
                mp = ps("pm", [1, 512], f"modp{g}_{n2}")
                for kc in range(TD // P):
                    nc.tensor.matmul(mp[:], silu8[:, kc:kc + 1], mw[:, kc, :],
                                     start=(kc == 0), stop=(kc == TD // P - 1))
                sl = slice(n2 * 512, (n2 + 1) * 512)
                nc.vector.tensor_add(mrow[:, sl], mrowf[:, sl], mp[:])
            return mrow

        def mod_to_cols(g, mrow, plus1):
            """Transpose a [1, D] mod row into per-feature cols [P, ND]."""
            if plus1:
                nc.vector.tensor_scalar_add(mrow[:], mrow[:], 1.0)
            col = consts.tile([P, ND], F32, tag=f"col{g}", name=f"col{g}")
            cp = psum.tile([P, 2 * ND], BF16, tag="pm", bufs=2,
                           name=f"colp{g}", padded_shape=[P, 512])
            for dc in range(ND):
                nc.tensor.transpose(cp[:, 2 * dc:2 * dc + 1],
                                    mrow[:, dc * P:(dc + 1) * P],
                                    ident_sb[0:1, 0:1])
            nc.vector.tensor_copy(col[:], cp[:].rearrange(
                "p (d two) -> p d two", two=2)[:, :, 0])
            return col

        shcol = mod_to_cols(0, mod_group(0, (modw_sb[(0, 0)], modw_sb[(0, 1)])),
                            False)
        sc1col = mod_to_cols(1, mod_group(1, (modw_sb[(1, 0)], modw_sb[(1, 1)])),
                             True)

        # ==============================================================
        # Stage 1: h = LN(x)*(scale+1)+shift ; hT = h.T
        # rstd via ACT Rsqrt (one op; rsqrt table stays loaded through
        # stage 1 + C since no exp runs in between).
        # ==============================================================
        for tt in range(NT):
            xt = x_tiles[tt]
            st = small.tile([P, 2, 6], F32, tag="bnst", bufs=2, name=f"st{tt}")
            nc.vector.bn_stats(st[:, 0, :], xt[:, 0:512])
            nc.vector.bn_stats(st[:, 1, :], xt[:, 512:1024])
            mv = small.tile([P, 2], F32, tag="bnmv", bufs=2, name=f"mv{tt}")
            nc.vector.bn_aggr(mv[:], st[:])
            sd = small.tile([P, 1], F32, tag="sd", bufs=2, name=f"sd{tt}")
            nc.scalar.activation(sd[:], mv[:, 1:2], AF.Sqrt, bias=eps_sb[:])
            rstd = small.tile([P, 1], F32, tag="rstd", bufs=2, name=f"rstd{tt}")
            nc.vector.reciprocal(rstd[:], sd[:])
            nmr = small.tile([P, 1], F32, tag="nmr", bufs=2, name=f"nmr{tt}")
            nc.vector.tensor_scalar(nmr[:], mv[:, 0:1], rstd[:], -1.0,
                                    OP.mult, OP.mult)
            xn = temps.tile([P, D], BF16, tag="xnb", bufs=2, name=f"xn{tt}")
            nc.scalar.activation(xn[:], xt[:], AF.Identity, bias=nmr[:],
                                 scale=rstd[:])
            # 4 transposes per [P,512] bf16 psum buf, fused affine evac
            for half in range(2):
                tp = psum.tile([P, 512], BF16, tag="pq", bufs=2,
                               name=f"tr{tt}_{half}", padded_shape=[P, 512])
                for q in range(4):
                    dc = half * 4 + q
                    nc.tensor.transpose(tp[:, q * P:(q + 1) * P],
                                        xn[:, dc * P:(dc + 1) * P], ident_sb[:])
                for q in range(4):
                    dc = half * 4 + q
                    if dc % 2 == 0:
                        nc.vector.tensor_scalar(
                            hT[:, dc, tt * P:(tt + 1) * P],
                            tp[:, q * P:(q + 1) * P],
                            sc1col[:, dc:dc + 1], shcol[:, dc:dc + 1],
                            OP.mult, OP.add)
                    else:
                        nc.scalar.activation(
                            hT[:, dc, tt * P:(tt + 1) * P],
                            tp[:, q * P:(q + 1) * P], AF.Identity,
                            bias=shcol[:, dc:dc + 1],
                            scale=sc1col[:, dc:dc + 1])

        # ==============================================================
        # Building blocks + filler-generator machinery.
        # Fillers are generators that emit ~2 independent matmuls per
        # next() ("quantum"); head_attention pumps them between tk
        # steps so the PE instruction stream has no gaps (keeps the
        # HAM clock gate at K=8/8 = 2.4GHz).
        # ==============================================================
        nc.gpsimd.memset(v_sb[:, :, :, E:E + 8], 0.0)
        for h in range(H):
            nc.gpsimd.memset(v_sb[:, :, h, E + h % 8], 1.0)

        filler = []

        def pump(n=1):
            done = 0
            while filler and done < n:
                try:
                    next(filler[0])
                    done += 1
                except StopIteration:
                    filler.pop(0)

        def drain():
            while filler:
                pump(1)

        def v_block(tt):
            vps = [ps("po", [P, 512], f"vp{tt}_{nv}") for nv in range(2)]
            for kc in range(ND):
                for nv in range(2):
                    nc.tensor.matmul(vps[nv][:], hT[:, kc, tt * P:(tt + 1) * P],
                                     wv_sb[nv][:, kc, :],
                                     start=(kc == 0), stop=(kc == ND - 1))
                pump(1)
            for nv in range(2):
                nc.scalar.copy(
                    v_sb[:, tt, nv * 8:(nv + 1) * 8, 0:E],
                    vps[nv][:].rearrange("p (h e) -> p h e", e=E))

        def gen_qk_chunk(jc):
            """Project w_qkv col block jc against hT -> qT/kT chunk."""
            which = 0 if jc < ND else 1
            dst = qT if which == 0 else kT
            jd = jc % ND
            wt = wqk_sb[jc]
            qps = [ps("pq", [P, 512], f"qp{jc}_{tn}") for tn in range(2)]
            for kc in range(ND):
                for tn in range(2):
                    nc.tensor.matmul(qps[tn][:], wt[:, kc, :],
                                     hT[:, kc, tn * 512:(tn + 1) * 512],
                                     start=(kc == 0), stop=(kc == ND - 1))
                yield
            for tn in range(2):
                nc.vector.tensor_copy(dst[:, jd, tn * 512:(tn + 1) * 512],
                                      qps[tn][:])

        def qk_chunk(jc):
            for _ in gen_qk_chunk(jc):
                pass

        stats_state = {}

        stats_work = {}

        def gen_stats(which, g):
            """LN stats for head group g: per-segment sum/sumsq matmuls,
            then mean/var math on DVE.  Variances land in one packed
            [8,4,512] tile so gen_stats_fin can do the whole group's
            rstd as ONE ln + ONE exp instruction (2 table switches,
            un-scatterable by the scheduler)."""
            src_t = qT if which == 0 else kT
            jts = range(4 * g, 4 * g + 4)
            if g not in stats_work:
                v8 = small.tile([8, 4, 512], F32, tag="var8", bufs=1,
                                name=f"var8_{g}")
                A = small.tile([8, 2, T], BF16, tag="Aall", bufs=1,
                               name=f"Aall_{g}")
                stats_work[g] = (v8, A, [])
            v8, A, mus = stats_work[g]
            Bt = small.tile([8, T], BF16, tag="Bsb", bufs=2,
                            name=f"B{which}_{g}")
            stats_state[(which, g)] = (A[:, which, :], Bt)
            for tn in range(2):
                sl = slice(tn * 512, (tn + 1) * 512)
                sqs = {}
                for jt in jts:
                    sq = temps.tile([P, 512], BF16, tag="sqt", bufs=4,
                                    name=f"sq{which}_{jt}_{tn}")
                    nc.vector.tensor_mul(sq[:], src_t[:, jt, sl],
                                         src_t[:, jt, sl])
                    sqs[jt] = sq
                yield
                stp = ps("pm", [P, 512], f"st{which}_{g}_{tn}")
                for i, jt in enumerate(jts):
                    es = eseg_sb[:, jt, 8 * g:8 * g + 8]
                    nc.tensor.matmul(stp[0:8, :], es, src_t[:, jt, sl],
                                     start=(i == 0), stop=(i == 3))
                    nc.tensor.matmul(stp[64:72, :], es, sqs[jt][:],
                                     start=(i == 0), stop=(i == 3))
                    if i == 1:
                        yield
                yield
                mu = small.tile([8, 512], F32, tag="scr2", bufs=4,
                                name=f"mu{which}_{g}_{tn}")
                nc.vector.tensor_scalar_mul(mu[:], stp[0:8, :], 1.0 / E)
                var = v8[:, 2 * which + tn, :]
                nc.vector.tensor_mul(var, mu[:], mu[:])
                nc.vector.scalar_tensor_tensor(var, stp[64:72, :],
                                               1.0 / E, var,
                                               OP.mult, OP.subtract)
                mus.append((which, tn, sl, mu))
                yield

        def gen_stats_fin(g):
            """rstd = exp(-0.5*ln(var+eps)) for the whole group as one
            wide ln + one wide exp, then the B = -mu*rstd math on DVE."""
            v8, A, mus = stats_work[g]
            nc.scalar.activation(v8[:], v8[:], AF.Ln, bias=eps_sb[0:8])
            yield
            aview = A.rearrange("r w (tn c) -> r (w tn) c", c=512)
            nc.scalar.activation(aview, v8[:], AF.Exp, scale=-0.5)
            for which, tn, sl, mu in mus:
                _, Bt = stats_state[(which, g)]
                nc.vector.scalar_tensor_tensor(Bt[:, sl], mu[:], -1.0,
                                               A[:, which, sl],
                                               OP.mult, OP.mult)
            yield

        def gen_rope(which, jt, g, pool=False):
            """src = rope(src*bcA + bcB) in place."""
            A, Bt = stats_state[(which, g)]
            src = (qT if which == 0 else kT)[:, jt, :]
            wcol = wq_sb if which == 0 else wk_sb
            eng = nc.gpsimd if pool else nc.vector
            bj = bsegj_sb[jt % 4]
            t2 = temps.tile([P, T], BF16, tag="ropet", bufs=2,
                            name=f"t2r{which}_{jt}")
            for tn in range(2):
                sl = slice(tn * 512, (tn + 1) * 512)
                bpA = ps("pm", [P, 512], f"bpA{which}_{jt}_{tn}")
                nc.tensor.matmul(bpA[:], bj[:], A[:, sl])
                bpB = ps("pq", [P, 512], f"bpB{which}_{jt}_{tn}")
                nc.tensor.matmul(bpB[:], bj[:], Bt[:, sl])
                nc.vector.tensor_mul(t2[:, sl], src[:, sl], bpA[:])
                nc.vector.tensor_add(t2[:, sl], t2[:, sl], bpB[:])
                yield
            if apply_qk_weight:
                nc.vector.tensor_scalar_mul(t2[:], t2[:], wcol[:])
            shf = temps.tile([P, T], BF16, tag="ropes", bufs=2,
                             name=f"shf{which}_{jt}")
            nc.vector.stream_shuffle(shf[:], t2[:], SWAP_MASK)
            eng.tensor_mul(src, t2[:], cs_sb[:])
            yield
            eng.tensor_mul(shf[:], shf[:], sn_sb[:])
            eng.tensor_add(src, src, shf[:])

        denA = [small.tile([8, T], F32, tag=f"denA{g}", name=f"denA{g}")
                for g in range(2)]
        for g in range(2):
            nc.vector.memset(denA[g][:], 0.0)
        rcp8 = [small.tile([8, T], BF16, tag=f"rcp8_{g}", name=f"rcp8_{g}")
                for g in range(2)]

        def den_recip(g):
            """rcp8[g] = 1/denA[g] via exp(-ln(x)) - same ACT table set."""
            lnd = small.tile([8, T], F32, tag="lnd", bufs=1, name=f"lnd{g}")
            nc.scalar.activation(lnd[:], denA[g][:], AF.Ln)
            nc.scalar.activation(rcp8[g][:], lnd[:], AF.Exp, scale=-1.0)

        def pair_attention(jc, rate2=True):
            """Both heads of chunk jc together, tn-major.  The two score
            matmuls contract over disjoint 64-partition halves of kT, so
            they land on disjoint PE row groups and run concurrently;
            one wide [P,1024] exp covers both heads\' tk block.  Pumps
            filler quanta between steps to keep the PE gap-free."""
            h0 = 2 * jc
            deng = denA[h0 // 8]
            for tn in range(2):
                sl = slice(tn * 512, (tn + 1) * 512)
                opsp = [ps("po", [E + 8, 512], f"o{h0}_{tn}_{i}")
                        for i in range(2)]

                def av(tk, ex):
                    for i in range(2):
                        nc.tensor.matmul(opsp[i][:],
                                         v_sb[:, tk, h0 + i, :],
                                         ex[:, i * 512:(i + 1) * 512],
                                         start=(tk == 0), stop=(tk == NT - 1))

                prev = None
                for tk in range(NT):
                    scw = psum.tile([P, 1024], F32, tag="sc", bufs=1,
                                    name=f"sc{h0}_{tn}_{tk}",
                                    padded_shape=[P, 1024])
                    for i in range(2):
                        nc.tensor.matmul(
                            scw[:, i * 512:(i + 1) * 512],
                            kT[i * E:(i + 1) * E, jc, tk * P:(tk + 1) * P],
                            qT[i * E:(i + 1) * E, jc, sl])
                    ex = temps.tile([P, T], BF16, tag="exp", bufs=3,
                                    name=f"ex{h0}_{tn}_{tk}")
                    nc.scalar.activation(ex[:], scw[:], AF.Exp, scale=0.125)
                    pump(1)
                    if prev is not None:
                        av(*prev)
                    if rate2:
                        pump(1)
                    prev = (tk, ex)
                av(*prev)
                for i in range(2):
                    p0 = i * E
                    nc.vector.tensor_copy(oTn[p0:p0 + E, jc, sl],
                                          opsp[i][0:E, :])
                    nc.vector.tensor_add(deng[:, sl], deng[:, sl],
                                         opsp[i][E:E + 8, :])

        def gen_norm(jt, rcp):
            bj = bsegj_sb[jt % 4]
            for tn in range(2):
                sl = slice(tn * 512, (tn + 1) * 512)
                br = ps("pm", [P, 512], f"brn{jt}_{tn}")
                nc.tensor.matmul(br[:], bj[:], rcp[:, sl])
                nc.vector.tensor_mul(oTn[:, jt, sl], oTn[:, jt, sl], br[:])
                yield

        def gen_y_block(tt, kcs, to_y0):
            """y partial for token tile tt over oTn chunks kcs."""
            yps = [ps("pq", [P, 512], f"yp{tt}_{len(kcs)}_{tn}")
                   for tn in range(2)]
            for i, kc in enumerate(kcs):
                for tn in range(2):
                    nc.tensor.matmul(yps[tn][:],
                                     oTn[:, kc, tt * P:(tt + 1) * P],
                                     wof[tn][:, kc, :],
                                     start=(i == 0), stop=(i == len(kcs) - 1))
                yield
            if to_y0:
                for tn in range(2):
                    nc.vector.tensor_copy(y0[:, tt, tn, :], yps[tn][:])
            else:
                y_sb = temps.tile([P, D], F32, tag="ysb", bufs=2,
                                  name=f"y{tt}")
                for tn in range(2):
                    sl = slice(tn * 512, (tn + 1) * 512)
                    nc.vector.scalar_tensor_tensor(y_sb[:, sl], yps[tn][:],
                                                   1.0, y0[:, tt, tn, :],
                                                   OP.mult, OP.add)
                    nc.vector.tensor_mul(y_sb[:, sl], y_sb[:, sl],
                                         gateB[:, sl])
                nc.sync.dma_start(out_e[tt * P:(tt + 1) * P, :], y_sb[:])

        def gen_mid_dmas():
            for n2 in range(2):
                mw = wbig_tile(f"modwg_{n2}")
                col0 = 2 * D + n2 * 512
                nc.sync.dma_start(
                    mw[:], modw_e[:, col0:col0 + 512].rearrange(
                        "(kc p) j -> p kc j", p=P))
                modw_sb[(2, n2)] = mw
            for tn in range(2):
                w = wbig_tile(f"wof{tn}")
                nc.sync.dma_start(
                    w[:], wout_e[:, tn * 512:(tn + 1) * 512].rearrange(
                        "(kc p) j -> p kc j", p=P))
                wof.append(w)
            yield

        def gen_gate():
            growb = mod_group(2, (modw_sb[(2, 0)], modw_sb[(2, 1)]))
            yield
            for n2 in range(2):
                gsl = slice(n2 * 512, (n2 + 1) * 512)
                bp = ps("pm", [P, 512], f"gbc{n2}")
                nc.tensor.matmul(bp[:], ones_sb[:], growb[:, gsl])
                nc.vector.tensor_copy(gateB[:, gsl], bp[:])
            yield

        wof = []
        gateB = consts.tile([P, D], BF16, tag="gateB", name="gateB")

        # ==============================================================
        # B: g0 projections (dense PE, warms the HAM clock gate)
        # ==============================================================
        for jc in g0_jcs:
            qk_chunk(jc)

        # ==============================================================
        # C: v blocks pumping g0 stats + ropes (rsqrt table loaded,
        # no exp in flight -> stats finish inline)
        # ==============================================================
        filler.append(gen_stats(0, 0))
        filler.append(gen_stats(1, 0))
        filler.append(gen_stats_fin(0))
        for jt in range(4):
            filler.append(gen_rope(0, jt, 0, pool=(jt % 2 == 1)))
            filler.append(gen_rope(1, jt, 0, pool=(jt % 2 == 0)))
        for tt in range(NT):
            v_block(tt)
        drain()

        # ==============================================================
        # P1: g0 attention pumping g1 projections -> stats -> ropes.
        # Non-exp ACT work (Rsqrt cluster, den recip) sits at head
        # boundaries to batch table switches.
        # ==============================================================
        filler.append(gen_mid_dmas())
        for jc in g1_jcs:
            filler.append(gen_qk_chunk(jc))
        filler.append(gen_stats(0, 1))
        filler.append(gen_stats(1, 1))
        filler.append(gen_stats_fin(1))
        filler.append(gen_gate())
        filler.append(gen_rope(0, 4, 1, pool=False))
        filler.append(gen_rope(1, 4, 1, pool=True))
        for jc in range(4):
            pair_attention(jc, rate2=False)
        drain()

        # ==============================================================
        # P2: g1 attention.  den0's recip is the first pumped quantum
        # (its inputs completed at P1 end), then ropes jt5-7 (needed by
        # pair 5+), then g0 normalize + g0 y0 partials behind them.
        # ==============================================================
        def gen_den0():
            den_recip(0)
            yield

        filler.append(gen_den0())
        for jt in range(5, 8):
            filler.append(gen_rope(0, jt, 1, pool=(jt % 2 == 1)))
            filler.append(gen_rope(1, jt, 1, pool=(jt % 2 == 0)))
        for jt in range(4):
            filler.append(gen_norm(jt, rcp8[0]))
        for tt in range(NT):
            filler.append(gen_y_block(tt, [0, 1, 2, 3], True))
        for jc in range(4, 8):
            pair_attention(jc)
        drain()
        den_recip(1)

        # ==============================================================
        # Tail: g1 normalize + g1 y partials + gate + out
        # ==============================================================
        for jt in range(4, 8):
            filler.append(gen_norm(jt, rcp8[1]))
        for tt in range(NT):
            filler.append(gen_y_block(tt, [4, 5, 6, 7], False))
        drain()


# =====================================================================
# Host side
# =====================================================================
_NC_CACHE = {}


def _get_nc(apply_qk_weight: bool):
    key = bool(apply_qk_weight)
    if key not in _NC_CACHE:
        _NC_CACHE[key] = build_nc(key)
    return _NC_CACHE[key]


def _make_consts(position, q_norm_w, k_norm_w):
    cs = np.ones((P, T), np.float32)
    sn = np.zeros((P, T), np.float32)
    cos = position[:, :, 0].T.astype(np.float32)   # [16, T]
    sin = position[:, :, 1].T.astype(np.float32)
    for half in (0, 64):
        for rr in range(32):
            j = rr // 2
            cs[half + rr, :] = cos[j]
            sn[half + rr, :] = sin[j] if (rr % 2 == 1) else -sin[j]
    eseg = np.zeros((P, ND, 16), np.float32)
    for t in range(ND):
        for p in range(P):
            eseg[p, t, 2 * t + p // E] = 1.0
    bsegj = np.zeros((8, 4, P), np.float32)
    for j in range(4):
        for p in range(P):
            bsegj[2 * j + p // 64, j, p] = 1.0
    import ml_dtypes  # noqa: deferred import keeps numpy-only callers fast
    return dict(
        cs_full=cs.astype(ml_dtypes.bfloat16), sn_full=sn.astype(ml_dtypes.bfloat16),
        eseg=eseg.astype(ml_dtypes.bfloat16),
        bsegj=bsegj.astype(ml_dtypes.bfloat16),
        ident=np.eye(P, dtype=np.float32).astype(ml_dtypes.bfloat16),
        ones_row=np.ones((1, P), np.float32).astype(ml_dtypes.bfloat16),
        wq_col=np.tile(q_norm_w.astype(np.float32), 2).reshape(P, 1),
        wk_col=np.tile(k_norm_w.astype(np.float32), 2).reshape(P, 1),
    )


def _prep_weights(mod_w, w_qkv, w_out):
    import ml_dtypes
    return dict(
        mod_w=np.ascontiguousarray(np.asarray(mod_w, np.float32)
                                   .astype(ml_dtypes.bfloat16)),
        w_qkv=np.ascontiguousarray(np.asarray(w_qkv, np.float32)
                                   .astype(ml_dtypes.bfloat16)),
        w_out=np.ascontiguousarray(np.asarray(w_out, np.float32)
                                   .astype(ml_dtypes.bfloat16)),
    )


def kernel(x, time, position, mod_w, mod_b, w_qkv, w_out, q_norm_w, k_norm_w):
    x = np.ascontiguousarray(np.asarray(x, dtype=np.float32))
    time = np.ascontiguousarray(np.asarray(time, dtype=np.float32))
    position = np.asarray(position, dtype=np.float32)
    mod_b = np.ascontiguousarray(np.asarray(mod_b, dtype=np.float32))
    q_norm_w = np.asarray(q_norm_w, dtype=np.float32)
    k_norm_w = np.asarray(k_norm_w, dtype=np.float32)
    wts = _prep_weights(mod_w, w_qkv, w_out)

    apply_w = not (np.all(q_norm_w == 1.0) and np.all(k_norm_w == 1.0))
    nc = _get_nc(apply_w)
    consts = _make_consts(position, q_norm_w, k_norm_w)

    in_maps = [
        dict(x=x[b], time=time[b].reshape(TD), mod_b=mod_b, **wts, **consts)
        for b in range(B)
    ]
    res = run_bass_kernel_spmd(nc, in_maps, core_ids=list(range(B)))
    out = np.stack([res.results[b]["out"] for b in range(B)], axis=0)
    return out.astype(np.float32)


if __name__ == "__main__":
    nc = build_nc(False)
    print("graph built ok")


# revision 30
# speedup vs baseline: 1.1650x; 1.1650x over previous
"""AdaLN-modulated multi-head attention block on 8 TRN2 NeuronCores.

Shapes (hardcoded): B=8, T=1024, D=1024, H=16 heads, e=64 head dim.
Sharding: pure data-parallel - one batch element per core, weights
replicated, no collectives.

v3 design (vs v2 baseline, 590us):
  The v2 trace showed PE busy 75% but with throttle_active=344us: the
  strict phase segregation (V-blocks with idle PE) kept the HAM clock
  gate at K=4/8 for most of the kernel, so matmuls ran at 1.2GHz.
  v3 software-pipelines across head groups so the PE always has
  independent matmul work queued:
    - v blocks + g0 stats/rope interleaved (C phase)
    - g0 attention interleaved with g1 projections/stats/ropes (P1)
    - g1 attention interleaved with g0 normalize + g0 y-partials (P2)
    - tail: g1 normalize + g1 y-partials + gate + out
  All ACT transcendentals use one table set (natural_log_exp):
    rsqrt(v) = exp(-0.5*ln(v+eps)); 1/x = exp(-ln(x));
    silu(x) = x * 1/(1+exp(-x)) (tiny DVE recip)
  This kills ~12 ACT table switches and ~35us of slow DVE reciprocals.
"""

import sys

try:
    import concourse  # noqa: F401  (provided by the environment, e.g. axon_site)
except ImportError:
    sys.path.append("/opt/trn_rl_repo")

import contextlib

import numpy as np

import concourse.bass as bass
import concourse.mybir as mybir
import concourse.tile as tile
from concourse import bacc
from concourse.bass_utils import run_bass_kernel_spmd

F32 = mybir.dt.float32
BF16 = mybir.dt.bfloat16
AF = mybir.ActivationFunctionType
OP = mybir.AluOpType

B, T, D, TD = 8, 1024, 1024, 1024
H, E = 16, 64
P = 128
NT = T // P          # 8 token tiles
ND = D // P          # 8 feature tiles
EPS = 1e-6
N3 = 3 * D
SWAP_MASK = [i ^ 1 for i in range(32)]
PATCH_ACT_TABLES = False


def _patch_act_tables():
    """Make Exp/Ln resolve to the combined natural_log_exp table set.

    The table-load pass maps each activation function to a covering
    set; by default Exp lands in exp_and_others and Ln in natural_log,
    so a kernel mixing them thrashes ACT_TABLE_LOADs (1.3us each).
    Filtering Exp/Ln out of every other set (set ids/order preserved)
    forces both onto natural_log_exp_and_others -> one load total.
    """
    if getattr(bacc, "_act_tables_patched", False):
        return
    orig = bacc.get_activation_tables

    def patched(arch):
        t = orig(arch)
        combo = t.get("natural_log_exp_and_others")
        if combo:
            drop = {f for f in (AF.Exp, AF.Ln) if f in combo}
            for name, fns in t.items():
                if name != "natural_log_exp_and_others":
                    fns.difference_update(drop)
        return t

    bacc.get_activation_tables = patched
    bacc._act_tables_patched = True


def build_nc(apply_qk_weight: bool):
    if PATCH_ACT_TABLES:
        _patch_act_tables()
    nc = bacc.Bacc("TRN2", target_bir_lowering=False, debug=False, num_devices=8)

    aps = {}
    aps["x"] = nc.dram_tensor("x", [T, D], F32, kind="ExternalInput").ap()
    aps["time"] = nc.dram_tensor("time", [TD], F32, kind="ExternalInput").ap()
    # weights arrive host-preconverted to bf16 (halves DMA, no on-chip casts)
    aps["mod_w"] = nc.dram_tensor("mod_w", [TD, N3], BF16, kind="ExternalInput").ap()
    aps["mod_b"] = nc.dram_tensor("mod_b", [N3], F32, kind="ExternalInput").ap()
    aps["w_qkv"] = nc.dram_tensor("w_qkv", [D, N3], BF16, kind="ExternalInput").ap()
    aps["w_out"] = nc.dram_tensor("w_out", [D, D], BF16, kind="ExternalInput").ap()
    # host-precomputed constants
    aps["cs_full"] = nc.dram_tensor("cs_full", [P, T], BF16, kind="ExternalInput").ap()
    aps["sn_full"] = nc.dram_tensor("sn_full", [P, T], BF16, kind="ExternalInput").ap()
    aps["eseg"] = nc.dram_tensor("eseg", [P, ND, 16], BF16, kind="ExternalInput").ap()
    aps["bsegj"] = nc.dram_tensor("bsegj", [8, 4, P], BF16,
                                  kind="ExternalInput").ap()
    aps["ident"] = nc.dram_tensor("ident", [P, P], BF16, kind="ExternalInput").ap()
    aps["ones_row"] = nc.dram_tensor("ones_row", [1, P], BF16, kind="ExternalInput").ap()
    aps["wq_col"] = nc.dram_tensor("wq_col", [P, 1], F32, kind="ExternalInput").ap()
    aps["wk_col"] = nc.dram_tensor("wk_col", [P, 1], F32, kind="ExternalInput").ap()

    aps["out"] = nc.dram_tensor("out", [T, D], F32, kind="ExternalOutput").ap()

    with tile.TileContext(nc) as tc:
        _body(nc, tc, aps, apply_qk_weight)
    nc.finalize()
    return nc


def _body(nc, tc, aps, apply_qk_weight):
    x_e, time_e, modw_e = aps["x"], aps["time"], aps["mod_w"]
    modb_e, wqkv_e, wout_e = aps["mod_b"], aps["w_qkv"], aps["w_out"]
    out_e = aps["out"]

    ctx = contextlib.ExitStack()
    with ctx:
        consts = ctx.enter_context(tc.tile_pool(name="consts", bufs=1))
        big = ctx.enter_context(tc.tile_pool(name="big", bufs=1))
        wstr = ctx.enter_context(tc.tile_pool(name="wstr", bufs=1))
        temps = ctx.enter_context(tc.tile_pool(name="temps", bufs=2))
        small = ctx.enter_context(tc.tile_pool(name="small", bufs=1))
        psum = ctx.enter_context(tc.tile_pool(name="psum", bufs=2, space="PSUM"))

        # PSUM budget (8 banks of [128 x 2KB]):
        #   sc: 2 bufs - score halves
        #   po: 2 bufs - o / v accumulation
        #   pq: 2 bufs - q/k projection chains, transposes, y blocks, bpB
        #   pm: 2 bufs - stats, bpA, mod, gate/norm bcasts
        def ps(tag, shape, name, dtype=F32):
            return psum.tile(shape, dtype, tag=tag, bufs=2, name=name,
                             padded_shape=[P, 512])

        # ---- constants into SBUF -------------------------------------
        def cload(key, shape, dtype, name):
            t = consts.tile(shape, dtype, tag=name, name=name)
            nc.sync.dma_start(t[:], aps[key])
            return t

        ident_sb = cload("ident", [P, P], BF16, "ident_sb")
        eps_sb = consts.tile([P, 1], F32, tag="eps_sb", name="eps_sb")
        nc.gpsimd.memset(eps_sb[:], EPS)

        # ---- big resident tensors ------------------------------------
        hT = big.tile([P, ND, T], BF16, tag="hT", name="hT")       # 16K/part
        qT = big.tile([P, ND, T], BF16, tag="qT", name="qT")       # 16K
        kT = big.tile([P, ND, T], BF16, tag="kT", name="kT")       # 16K
        v_sb = big.tile([P, NT, H, E + 8], BF16, tag="v", name="v_sb")  # 18K
        y0 = big.tile([P, NT, 2, 512], BF16, tag="y0", name="y0")  # 16K
        oTn = qT   # head rows of qT are dead once that head's scores ran

        # ==============================================================
        # DMA prefetch order == PE consumption order:
        # modw(shift+scale) -> x -> wqk g0 -> wv -> wqk g1;
        # gate modw + w_out issued mid-P1.
        # ==============================================================
        def wbig_tile(name):
            return wstr.tile([P, ND, 512], BF16, tag="wbig", bufs=2, name=name)

        # Single sync DMA queue, ordered by need: x first (stage-1
        # streams over it), then mod_w (affine cols), qk weights for B,
        # v weights and rope/stats constants for C, then g1 weights.
        x_tiles = []
        for tt in range(NT):
            xt = temps.tile([P, D], F32, tag="xt", bufs=2, name=f"xt{tt}")
            nc.sync.dma_start(xt[:], x_e[tt * P:(tt + 1) * P, :])
            x_tiles.append(xt)

        modw_sb = {}
        for g, n2 in ((0, 0), (0, 1), (1, 0), (1, 1)):
            mw = wbig_tile(f"modw{g}_{n2}")
            col0 = g * D + n2 * 512
            nc.sync.dma_start(
                mw[:], modw_e[:, col0:col0 + 512].rearrange(
                    "(kc p) j -> p kc j", p=P))
            modw_sb[(g, n2)] = mw

        t8 = small.tile([P, TD // P], F32, tag="t8", name="t8")
        nc.sync.dma_start(t8[:], time_e.rearrange("(o p) -> p o", p=P))
        mrowf_sb = []
        for g in range(2):
            mrowf = temps.tile([1, D], F32, tag="rbc", bufs=2,
                               name=f"mrowf{g}")
            nc.sync.dma_start(mrowf[:], modb_e[None, g * D:(g + 1) * D])
            mrowf_sb.append(mrowf)

        g0_jcs = [jc for jt in range(4) for jc in (jt, ND + jt)]
        g1_jcs = [jc for jt in range(4, 8) for jc in (jt, ND + jt)]
        wqk_sb = {}
        for jc in g0_jcs:
            wt = wstr.tile([P, ND, P], BF16, tag="wqk", bufs=6,
                           name=f"wqk{jc}")
            nc.sync.dma_start(
                wt[:], wqkv_e[:, jc * P:(jc + 1) * P].rearrange(
                    "(kc p) j -> p kc j", p=P))
            wqk_sb[jc] = wt

        wv_sb = []
        for nv in range(2):
            wv = wbig_tile(f"wv{nv}")
            nc.sync.dma_start(
                wv[:], wqkv_e[:, 2048 + nv * 512:2048 + (nv + 1) * 512]
                .rearrange("(kc p) j -> p kc j", p=P))
            wv_sb.append(wv)

        cs_sb = cload("cs_full", [P, T], BF16, "cs_sb")
        sn_sb = cload("sn_full", [P, T], BF16, "sn_sb")
        eseg_sb = cload("eseg", [P, ND, 16], BF16, "eseg_sb")
        bsegj_all = cload("bsegj", [8, 4, P], BF16, "bsegj_sb")
        bsegj_sb = [bsegj_all[:, j, :] for j in range(4)]
        ones_sb = cload("ones_row", [1, P], BF16, "ones_sb")
        wq_sb = cload("wq_col", [P, 1], F32, "wq_sb")
        wk_sb = cload("wk_col", [P, 1], F32, "wk_sb")

        for jc in g1_jcs:
            wt = wstr.tile([P, ND, P], BF16, tag="wqk", bufs=6,
                           name=f"wqk{jc}")
            nc.sync.dma_start(
                wt[:], wqkv_e[:, jc * P:(jc + 1) * P].rearrange(
                    "(kc p) j -> p kc j", p=P))
            wqk_sb[jc] = wt

        # ==============================================================
        # Stage 0: mod = silu(time) @ mod_w + mod_b (shift/scale now).
        # silu = x * 1/(1+exp(-x)) - stays in the exp table set.
        # ==============================================================
        ex8 = small.tile([P, TD // P], F32, tag="ex8", name="ex8")
        nc.scalar.activation(ex8[:], t8[:], AF.Exp, scale=-1.0)
        nc.vector.tensor_scalar_add(ex8[:], ex8[:], 1.0)
        r8 = small.tile([P, TD // P], F32, tag="r8", name="r8")
        nc.vector.reciprocal(r8[:], ex8[:])
        silu8 = small.tile([P, TD // P], BF16, tag="silu8", name="silu8")
        nc.vector.tensor_mul(silu8[:], t8[:], r8[:])

        def mod_group(g, mw_pair):
            """Compute mod group g -> [1, D] bf16 row (bias added)."""
            if g < 2:
                mrowf = mrowf_sb[g]
            else:
                mrowf = temps.tile([1, D], F32, tag="rbc", bufs=2,
                                   name=f"mrowf{g}")
                nc.sync.dma_start(mrowf[:], modb_e[None, g * D:(g + 1) * D])
            mrow = small.tile([1, D], BF16, tag="mrow", bufs=1,
                              name=f"mrow{g}")
            for n2 in range(2):
                mw = mw_pair[n2]
                mp = ps("pm", [1, 512], f"modp{g}_{n2}")
                for kc in range(TD // P):
                    nc.tensor.matmul(mp[:], silu8[:, kc:kc + 1], mw[:, kc, :],
                                     start=(kc == 0), stop=(kc == TD // P - 1))
                sl = slice(n2 * 512, (n2 + 1) * 512)
                nc.vector.tensor_add(mrow[:, sl], mrowf[:, sl], mp[:])
            return mrow

        def mod_to_cols(g, mrow, plus1):
            """Transpose a [1, D] mod row into per-feature cols [P, ND]."""
            if plus1:
                nc.vector.tensor_scalar_add(mrow[:], mrow[:], 1.0)
            col = consts.tile([P, ND], F32, tag=f"col{g}", name=f"col{g}")
            cp = psum.tile([P, 2 * ND], BF16, tag="pm", bufs=2,
                           name=f"colp{g}", padded_shape=[P, 512])
            for dc in range(ND):
                nc.tensor.transpose(cp[:, 2 * dc:2 * dc + 1],
                                    mrow[:, dc * P:(dc + 1) * P],
                                    ident_sb[0:1, 0:1])
            nc.vector.tensor_copy(col[:], cp[:].rearrange(
                "p (d two) -> p d two", two=2)[:, :, 0])
            return col

        shcol = mod_to_cols(0, mod_group(0, (modw_sb[(0, 0)], modw_sb[(0, 1)])),
                            False)
        sc1col = mod_to_cols(1, mod_group(1, (modw_sb[(1, 0)], modw_sb[(1, 1)])),
                             True)

        # ==============================================================
        # Stage 1: h = LN(x)*(scale+1)+shift ; hT = h.T
        # rstd via ACT Rsqrt (one op; rsqrt table stays loaded through
        # stage 1 + C since no exp runs in between).
        # ==============================================================
        for tt in range(NT):
            xt = x_tiles[tt]
            st = small.tile([P, 2, 6], F32, tag="bnst", bufs=2, name=f"st{tt}")
            nc.vector.bn_stats(st[:, 0, :], xt[:, 0:512])
            nc.vector.bn_stats(st[:, 1, :], xt[:, 512:1024])
            mv = small.tile([P, 2], F32, tag="bnmv", bufs=2, name=f"mv{tt}")
            nc.vector.bn_aggr(mv[:], st[:])
            sd = small.tile([P, 1], F32, tag="sd", bufs=2, name=f"sd{tt}")
            nc.scalar.activation(sd[:], mv[:, 1:2], AF.Sqrt, bias=eps_sb[:])
            rstd = small.tile([P, 1], F32, tag="rstd", bufs=2, name=f"rstd{tt}")
            nc.vector.reciprocal(rstd[:], sd[:])
            nmr = small.tile([P, 1], F32, tag="nmr", bufs=2, name=f"nmr{tt}")
            nc.vector.tensor_scalar(nmr[:], mv[:, 0:1], rstd[:], -1.0,
                                    OP.mult, OP.mult)
            xn = temps.tile([P, D], BF16, tag="xnb", bufs=2, name=f"xn{tt}")
            nc.scalar.activation(xn[:], xt[:], AF.Identity, bias=nmr[:],
                                 scale=rstd[:])
            # 4 transposes per [P,512] bf16 psum buf, fused affine evac
            for half in range(2):
                tp = psum.tile([P, 512], BF16, tag="pq", bufs=2,
                               name=f"tr{tt}_{half}", padded_shape=[P, 512])
                for q in range(4):
                    dc = half * 4 + q
                    nc.tensor.transpose(tp[:, q * P:(q + 1) * P],
                                        xn[:, dc * P:(dc + 1) * P], ident_sb[:])
                for q in range(4):
                    dc = half * 4 + q
                    if dc % 2 == 0:
                        nc.vector.tensor_scalar(
                            hT[:, dc, tt * P:(tt + 1) * P],
                            tp[:, q * P:(q + 1) * P],
                            sc1col[:, dc:dc + 1], shcol[:, dc:dc + 1],
                            OP.mult, OP.add)
                    else:
                        nc.scalar.activation(
                            hT[:, dc, tt * P:(tt + 1) * P],
                            tp[:, q * P:(q + 1) * P], AF.Identity,
                            bias=shcol[:, dc:dc + 1],
                            scale=sc1col[:, dc:dc + 1])

        # ==============================================================
        # Building blocks + filler-generator machinery.
        # Fillers are generators that emit ~2 independent matmuls per
        # next() ("quantum"); head_attention pumps them between tk
        # steps so the PE instruction stream has no gaps (keeps the
        # HAM clock gate at K=8/8 = 2.4GHz).
        # ==============================================================
        nc.gpsimd.memset(v_sb[:, :, :, E:E + 8], 0.0)
        for h in range(H):
            nc.gpsimd.memset(v_sb[:, :, h, E + h % 8], 1.0)

        filler = []

        def pump(n=1):
            done = 0
            while filler and done < n:
                try:
                    next(filler[0])
                    done += 1
                except StopIteration:
                    filler.pop(0)

        def drain():
            while filler:
                pump(1)

        def v_block(tt):
            vps = [ps("po", [P, 512], f"vp{tt}_{nv}") for nv in range(2)]
            for kc in range(ND):
                for nv in range(2):
                    nc.tensor.matmul(vps[nv][:], hT[:, kc, tt * P:(tt + 1) * P],
                                     wv_sb[nv][:, kc, :],
                                     start=(kc == 0), stop=(kc == ND - 1))
                pump(1)
            for nv in range(2):
                nc.scalar.copy(
                    v_sb[:, tt, nv * 8:(nv + 1) * 8, 0:E],
                    vps[nv][:].rearrange("p (h e) -> p h e", e=E))

        def gen_qk_chunk(jc):
            """Project w_qkv col block jc against hT -> qT/kT chunk."""
            which = 0 if jc < ND else 1
            dst = qT if which == 0 else kT
            jd = jc % ND
            wt = wqk_sb[jc]
            qps = [ps("pq", [P, 512], f"qp{jc}_{tn}") for tn in range(2)]
            for kc in range(ND):
                for tn in range(2):
                    nc.tensor.matmul(qps[tn][:], wt[:, kc, :],
                                     hT[:, kc, tn * 512:(tn + 1) * 512],
                                     start=(kc == 0), stop=(kc == ND - 1))
                yield
            for tn in range(2):
                nc.vector.tensor_copy(dst[:, jd, tn * 512:(tn + 1) * 512],
                                      qps[tn][:])

        def qk_chunk(jc):
            for _ in gen_qk_chunk(jc):
                pass

        stats_state = {}

        stats_work = {}

        def gen_stats(which, g):
            """LN stats for head group g: per-segment sum/sumsq matmuls,
            then mean/var math on DVE.  Variances land in one packed
            [8,4,512] tile so gen_stats_fin can do the whole group's
            rstd as ONE ln + ONE exp instruction (2 table switches,
            un-scatterable by the scheduler)."""
            src_t = qT if which == 0 else kT
            jts = range(4 * g, 4 * g + 4)
            if g not in stats_work:
                v8 = small.tile([8, 4, 512], F32, tag="var8", bufs=1,
                                name=f"var8_{g}")
                A = small.tile([8, 2, T], BF16, tag="Aall", bufs=1,
                               name=f"Aall_{g}")
                stats_work[g] = (v8, A, [])
            v8, A, mus = stats_work[g]
            Bt = small.tile([8, T], BF16, tag="Bsb", bufs=2,
                            name=f"B{which}_{g}")
            stats_state[(which, g)] = (A[:, which, :], Bt)
            for tn in range(2):
                sl = slice(tn * 512, (tn + 1) * 512)
                sqs = {}
                for jt in jts:
                    sq = temps.tile([P, 512], BF16, tag="sqt", bufs=4,
                                    name=f"sq{which}_{jt}_{tn}")
                    nc.vector.tensor_mul(sq[:], src_t[:, jt, sl],
                                         src_t[:, jt, sl])
                    sqs[jt] = sq
                yield
                stp = ps("pm", [P, 512], f"st{which}_{g}_{tn}")
                for i, jt in enumerate(jts):
                    es = eseg_sb[:, jt, 8 * g:8 * g + 8]
                    nc.tensor.matmul(stp[0:8, :], es, src_t[:, jt, sl],
                                     start=(i == 0), stop=(i == 3))
                    nc.tensor.matmul(stp[64:72, :], es, sqs[jt][:],
                                     start=(i == 0), stop=(i == 3))
                    if i == 1:
                        yield
                yield
                mu = small.tile([8, 512], F32, tag="scr2", bufs=4,
                                name=f"mu{which}_{g}_{tn}")
                nc.vector.tensor_scalar_mul(mu[:], stp[0:8, :], 1.0 / E)
                var = v8[:, 2 * which + tn, :]
                nc.vector.tensor_mul(var, mu[:], mu[:])
                nc.vector.scalar_tensor_tensor(var, stp[64:72, :],
                                               1.0 / E, var,
                                               OP.mult, OP.subtract)
                mus.append((which, tn, sl, mu))
                yield

        def gen_stats_fin(g):
            """rstd = exp(-0.5*ln(var+eps)) for the whole group as one
            wide ln + one wide exp, then the B = -mu*rstd math on DVE."""
            v8, A, mus = stats_work[g]
            nc.scalar.activation(v8[:], v8[:], AF.Ln, bias=eps_sb[0:8])
            yield
            aview = A.rearrange("r w (tn c) -> r (w tn) c", c=512)
            nc.scalar.activation(aview, v8[:], AF.Exp, scale=-0.5)
            for which, tn, sl, mu in mus:
                _, Bt = stats_state[(which, g)]
                nc.vector.scalar_tensor_tensor(Bt[:, sl], mu[:], -1.0,
                                               A[:, which, sl],
                                               OP.mult, OP.mult)
            yield

        def gen_rope(which, jt, g, pool=False):
            """src = rope(src*bcA + bcB) in place."""
            A, Bt = stats_state[(which, g)]
            src = (qT if which == 0 else kT)[:, jt, :]
            wcol = wq_sb if which == 0 else wk_sb
            eng = nc.gpsimd if pool else nc.vector
            bj = bsegj_sb[jt % 4]
            t2 = temps.tile([P, T], BF16, tag="ropet", bufs=2,
                            name=f"t2r{which}_{jt}")
            for tn in range(2):
                sl = slice(tn * 512, (tn + 1) * 512)
                bpA = ps("pm", [P, 512], f"bpA{which}_{jt}_{tn}")
                nc.tensor.matmul(bpA[:], bj[:], A[:, sl])
                bpB = ps("pq", [P, 512], f"bpB{which}_{jt}_{tn}")
                nc.tensor.matmul(bpB[:], bj[:], Bt[:, sl])
                nc.vector.tensor_mul(t2[:, sl], src[:, sl], bpA[:])
                nc.vector.tensor_add(t2[:, sl], t2[:, sl], bpB[:])
                yield
            if apply_qk_weight:
                nc.vector.tensor_scalar_mul(t2[:], t2[:], wcol[:])
            shf = temps.tile([P, T], BF16, tag="ropes", bufs=2,
                             name=f"shf{which}_{jt}")
            nc.vector.stream_shuffle(shf[:], t2[:], SWAP_MASK)
            eng.tensor_mul(src, t2[:], cs_sb[:])
            yield
            eng.tensor_mul(shf[:], shf[:], sn_sb[:])
            eng.tensor_add(src, src, shf[:])

        denA = [small.tile([8, T], F32, tag=f"denA{g}", name=f"denA{g}")
                for g in range(2)]
        for g in range(2):
            nc.vector.memset(denA[g][:], 0.0)
        rcp8 = [small.tile([8, T], BF16, tag=f"rcp8_{g}", name=f"rcp8_{g}")
                for g in range(2)]

        def den_recip(g):
            """rcp8[g] = 1/denA[g] via exp(-ln(x)) - same ACT table set."""
            lnd = small.tile([8, T], F32, tag="lnd", bufs=1, name=f"lnd{g}")
            nc.scalar.activation(lnd[:], denA[g][:], AF.Ln)
            nc.scalar.activation(rcp8[g][:], lnd[:], AF.Exp, scale=-1.0)

        def pair_attention(jc, rate2=True, evac_act=False):
            """Both heads of chunk jc together, tn-major.  The two score
            matmuls contract over disjoint 64-partition halves of kT, so
            they land on disjoint PE row groups and run concurrently;
            one wide [P,1024] exp covers both heads\' tk block.  Pumps
            filler quanta between steps to keep the PE gap-free."""
            h0 = 2 * jc
            deng = denA[h0 // 8]
            for tn in range(2):
                sl = slice(tn * 512, (tn + 1) * 512)
                opsp = [ps("po", [E + 8, 512], f"o{h0}_{tn}_{i}")
                        for i in range(2)]

                def av(tk, ex):
                    for i in range(2):
                        nc.tensor.matmul(opsp[i][:],
                                         v_sb[:, tk, h0 + i, :],
                                         ex[:, i * 512:(i + 1) * 512],
                                         start=(tk == 0), stop=(tk == NT - 1))

                prev = None
                for tk in range(NT):
                    scw = psum.tile([P, 1024], F32, tag="sc", bufs=1,
                                    name=f"sc{h0}_{tn}_{tk}",
                                    padded_shape=[P, 1024])
                    for i in range(2):
                        nc.tensor.matmul(
                            scw[:, i * 512:(i + 1) * 512],
                            kT[i * E:(i + 1) * E, jc, tk * P:(tk + 1) * P],
                            qT[i * E:(i + 1) * E, jc, sl])
                    ex = temps.tile([P, T], BF16, tag="exp", bufs=3,
                                    name=f"ex{h0}_{tn}_{tk}")
                    nc.scalar.activation(ex[:], scw[:], AF.Exp, scale=0.125)
                    pump(1)
                    if prev is not None:
                        av(*prev)
                    if rate2:
                        pump(1)
                    prev = (tk, ex)
                av(*prev)
                for i in range(2):
                    p0 = i * E
                    if evac_act:
                        nc.scalar.copy(oTn[p0:p0 + E, jc, sl],
                                       opsp[i][0:E, :])
                    else:
                        nc.vector.tensor_copy(oTn[p0:p0 + E, jc, sl],
                                              opsp[i][0:E, :])
                    nc.vector.tensor_add(deng[:, sl], deng[:, sl],
                                         opsp[i][E:E + 8, :])

        def gen_norm(jt, rcp):
            bj = bsegj_sb[jt % 4]
            for tn in range(2):
                sl = slice(tn * 512, (tn + 1) * 512)
                br = ps("pm", [P, 512], f"brn{jt}_{tn}")
                nc.tensor.matmul(br[:], bj[:], rcp[:, sl])
                nc.vector.tensor_mul(oTn[:, jt, sl], oTn[:, jt, sl], br[:])
                yield

        def gen_y_block(tt, kcs, to_y0):
            """y partial for token tile tt over oTn chunks kcs."""
            yps = [ps("pq", [P, 512], f"yp{tt}_{len(kcs)}_{tn}")
                   for tn in range(2)]
            for i, kc in enumerate(kcs):
                for tn in range(2):
                    nc.tensor.matmul(yps[tn][:],
                                     oTn[:, kc, tt * P:(tt + 1) * P],
                                     wof[tn][:, kc, :],
                                     start=(i == 0), stop=(i == len(kcs) - 1))
                yield
            if to_y0:
                for tn in range(2):
                    nc.vector.tensor_copy(y0[:, tt, tn, :], yps[tn][:])
            else:
                y_sb = temps.tile([P, D], F32, tag="ysb", bufs=2,
                                  name=f"y{tt}")
                for tn in range(2):
                    sl = slice(tn * 512, (tn + 1) * 512)
                    nc.vector.scalar_tensor_tensor(y_sb[:, sl], yps[tn][:],
                                                   1.0, y0[:, tt, tn, :],
                                                   OP.mult, OP.add)
                    nc.vector.tensor_mul(y_sb[:, sl], y_sb[:, sl],
                                         gateB[:, sl])
                nc.sync.dma_start(out_e[tt * P:(tt + 1) * P, :], y_sb[:])

        def gen_mid_dmas():
            for n2 in range(2):
                mw = wbig_tile(f"modwg_{n2}")
                col0 = 2 * D + n2 * 512
                nc.sync.dma_start(
                    mw[:], modw_e[:, col0:col0 + 512].rearrange(
                        "(kc p) j -> p kc j", p=P))
                modw_sb[(2, n2)] = mw
            for tn in range(2):
                w = wbig_tile(f"wof{tn}")
                nc.sync.dma_start(
                    w[:], wout_e[:, tn * 512:(tn + 1) * 512].rearrange(
                        "(kc p) j -> p kc j", p=P))
                wof.append(w)
            yield

        def gen_gate():
            growb = mod_group(2, (modw_sb[(2, 0)], modw_sb[(2, 1)]))
            yield
            for n2 in range(2):
                gsl = slice(n2 * 512, (n2 + 1) * 512)
                bp = ps("pm", [P, 512], f"gbc{n2}")
                nc.tensor.matmul(bp[:], ones_sb[:], growb[:, gsl])
                nc.vector.tensor_copy(gateB[:, gsl], bp[:])
            yield

        wof = []
        gateB = consts.tile([P, D], BF16, tag="gateB", name="gateB")

        # ==============================================================
        # B: g0 projections (dense PE, warms the HAM clock gate)
        # ==============================================================
        for jc in g0_jcs:
            qk_chunk(jc)

        # ==============================================================
        # C: v blocks pumping g0 stats + ropes (rsqrt table loaded,
        # no exp in flight -> stats finish inline)
        # ==============================================================
        filler.append(gen_stats(0, 0))
        filler.append(gen_stats(1, 0))
        filler.append(gen_stats_fin(0))
        for jt in range(4):
            filler.append(gen_rope(0, jt, 0, pool=(jt % 2 == 1)))
            filler.append(gen_rope(1, jt, 0, pool=(jt % 2 == 0)))
        for tt in range(NT):
            v_block(tt)
        drain()

        # ==============================================================
        # P1: g0 attention pumping g1 projections -> stats -> ropes.
        # Non-exp ACT work (Rsqrt cluster, den recip) sits at head
        # boundaries to batch table switches.
        # ==============================================================
        filler.append(gen_mid_dmas())
        for jc in g1_jcs:
            filler.append(gen_qk_chunk(jc))
        filler.append(gen_stats(0, 1))
        filler.append(gen_stats(1, 1))
        filler.append(gen_stats_fin(1))
        filler.append(gen_gate())
        filler.append(gen_rope(0, 4, 1, pool=True))
        filler.append(gen_rope(1, 4, 1, pool=True))
        for jc in range(4):
            pair_attention(jc, rate2=False)
        drain()

        # ==============================================================
        # P2: g1 attention.  Ropes jt5-7 (Pool-side trios - DVE is the
        # scarce engine here) pump during pair 4; den0's recip emits
        # after pair 4 when its inputs are long done; normalize then y0
        # partials pump behind it.  o-evacs ride ACT in P2.
        # ==============================================================
        for jt in range(5, 8):
            filler.append(gen_rope(0, jt, 1, pool=True))
            filler.append(gen_rope(1, jt, 1, pool=True))
        pair_attention(4, evac_act=True)
        den_recip(0)
        for jt in range(4):
            filler.append(gen_norm(jt, rcp8[0]))
        pair_attention(5, evac_act=True)
        for tt in range(NT):
            filler.append(gen_y_block(tt, [0, 1, 2, 3], True))
        pair_attention(6, evac_act=True)
        pair_attention(7, evac_act=True)
        drain()
        den_recip(1)

        # ==============================================================
        # Tail: g1 normalize + g1 y partials + gate + out
        # ==============================================================
        for jt in range(4, 8):
            filler.append(gen_norm(jt, rcp8[1]))
        for tt in range(NT):
            filler.append(gen_y_block(tt, [4, 5, 6, 7], False))
        drain()


# =====================================================================
# Host side
# =====================================================================
_NC_CACHE = {}


def _get_nc(apply_qk_weight: bool):
    key = bool(apply_qk_weight)
    if key not in _NC_CACHE:
        _NC_CACHE[key] = build_nc(key)
    return _NC_CACHE[key]


def _make_consts(position, q_norm_w, k_norm_w):
    cs = np.ones((P, T), np.float32)
    sn = np.zeros((P, T), np.float32)
    cos = position[:, :, 0].T.astype(np.float32)   # [16, T]
    sin = position[:, :, 1].T.astype(np.float32)
    for half in (0, 64):
        for rr in range(32):
            j = rr // 2
            cs[half + rr, :] = cos[j]
            sn[half + rr, :] = sin[j] if (rr % 2 == 1) else -sin[j]
    eseg = np.zeros((P, ND, 16), np.float32)
    for t in range(ND):
        for p in range(P):
            eseg[p, t, 2 * t + p // E] = 1.0
    bsegj = np.zeros((8, 4, P), np.float32)
    for j in range(4):
        for p in range(P):
            bsegj[2 * j + p // 64, j, p] = 1.0
    import ml_dtypes  # noqa: deferred import keeps numpy-only callers fast
    return dict(
        cs_full=cs.astype(ml_dtypes.bfloat16), sn_full=sn.astype(ml_dtypes.bfloat16),
        eseg=eseg.astype(ml_dtypes.bfloat16),
        bsegj=bsegj.astype(ml_dtypes.bfloat16),
        ident=np.eye(P, dtype=np.float32).astype(ml_dtypes.bfloat16),
        ones_row=np.ones((1, P), np.float32).astype(ml_dtypes.bfloat16),
        wq_col=np.tile(q_norm_w.astype(np.float32), 2).reshape(P, 1),
        wk_col=np.tile(k_norm_w.astype(np.float32), 2).reshape(P, 1),
    )


def _prep_weights(mod_w, w_qkv, w_out):
    import ml_dtypes
    return dict(
        mod_w=np.ascontiguousarray(np.asarray(mod_w, np.float32)
                                   .astype(ml_dtypes.bfloat16)),
        w_qkv=np.ascontiguousarray(np.asarray(w_qkv, np.float32)
                                   .astype(ml_dtypes.bfloat16)),
        w_out=np.ascontiguousarray(np.asarray(w_out, np.float32)
                                   .astype(ml_dtypes.bfloat16)),
    )


def kernel(x, time, position, mod_w, mod_b, w_qkv, w_out, q_norm_w, k_norm_w):
    x = np.ascontiguousarray(np.asarray(x, dtype=np.float32))
    time = np.ascontiguousarray(np.asarray(time, dtype=np.float32))
    position = np.asarray(position, dtype=np.float32)
    mod_b = np.ascontiguousarray(np.asarray(mod_b, dtype=np.float32))
    q_norm_w = np.asarray(q_norm_w, dtype=np.float32)
    k_norm_w = np.asarray(k_norm_w, dtype=np.float32)
    wts = _prep_weights(mod_w, w_qkv, w_out)

    apply_w = not (np.all(q_norm_w == 1.0) and np.all(k_norm_w == 1.0))
    nc = _get_nc(apply_w)
    consts = _make_consts(position, q_norm_w, k_norm_w)

    in_maps = [
        dict(x=x[b], time=time[b].reshape(TD), mod_b=mod_b, **wts, **consts)
        for b in range(B)
    ]
    res = run_bass_kernel_spmd(nc, in_maps, core_ids=list(range(B)))
    out = np.stack([res.results[b]["out"] for b in range(B)], axis=0)
    return out.astype(np.float32)


if __name__ == "__main__":
    nc = build_nc(False)
    print("graph built ok")
